# revision 20
# baseline (speedup 1.0000x reference)
"""Trainium2 Bass kernel for nn_ASD_72069551227061 (GNN message passing).

Strategy (8 NeuronCores, dst-sharded graph parallelism, fully on-device GCN):
  Each of the 5 GCN layers runs as ONE NEFF launch per layer; the NEFF does,
  per core (50k-dst shard, 391 windows of 128 dsts):
    - indirect-DMA gather of the 6.4M/8 edge messages from a replicated
      [N,70] fp16 send table (pro|lig packed halves),
    - segment-sum by destination via one-hot S-matrix matmuls in PSUM,
      where the one-hots are GENERATED ON DEVICE (iota + is_equal) instead
      of streamed from HBM (kills the 218MB/layer smat traffic of v1),
    - the self-loop term via a diag-scaled identity matmul,
    - the dst-side dinv scaling on PSUM eviction (per-partition scale),
    - the dense 35x35 GCN weight matmul + bias + ReLU on the PE (bias via a
      ones-row in the transposed activations),
    - the next layer's send table (spro/slig-scaled h), re-replicated across
      the 8 cores by a gpsimd AllGather INSIDE the NEFF,
    - batch pooling (one-hot over batch ids) accumulated in PSUM.
  Host work between launches is zero: layer k+1's inputs are layer k's
  device-resident outputs. Host does degrees/scales/table0 prep (untimed)
  and the tiny [128,...] BiLSTM/attention/MLP tail in fp32.
"""
import sys
sys.path.insert(0, "/opt/trn_rl_repo")

import numpy as np
import ml_dtypes

N, E, B, D, T, SEQ = 400_000, 6_400_000, 128, 35, 140, 2
F = 2 * D                    # 70: packed [pro | lig] feature row
FK = F + 1                   # 71: + ones row for bias
NCORES = 8
SHARD = N // NCORES          # 50_000 dsts per core
WIN = 128                    # dsts per PSUM window
NWIN = (SHARD + WIN - 1) // WIN   # 391 windows
PADN = NWIN * WIN            # 50_048 padded dst rows per core
NLAYER = 5

_CACHE = {}


def _mask_mat():
    m = np.eye(T, dtype=bool)
    m[SEQ:, :] = False
    m[:, SEQ:] = False
    m[:, SEQ - 1] = True
    m[SEQ - 1, :] = True
    m[SEQ - 1, SEQ - 1] = False
    return m


def _build_device_program(n_tiles_per_win, tw_list=None):
    import concourse.bass as bass
    import concourse.bacc as bacc
    import concourse.mybir as mybir
    import concourse.tile as tile

    TW = n_tiles_per_win
    if tw_list is None:
        tw_list = [TW] * NWIN
    CD = TW  # faux col offsets: dloc cols [0:TW], then scales
    nc = bacc.Bacc("TRN2", target_bir_lowering=False, debug=False, num_devices=NCORES)
    NP = NCORES * PADN
    table = nc.dram_tensor("table", [NP, F], mybir.dt.float16, kind="ExternalInput")
    hst = nc.dram_tensor("hst", [PADN, F], mybir.dt.float16, kind="ExternalInput")
    sidx = nc.dram_tensor("sidx", [NWIN, 128, TW], mybir.dt.int32, kind="ExternalInput")
    faux = nc.dram_tensor("faux", [NWIN, 128, TW + 5], mybir.dt.float32, kind="ExternalInput")
    wmat = nc.dram_tensor("wmat", [FK, F], mybir.dt.float16, kind="ExternalInput")
    tnext = nc.dram_tensor("tnext", [NP, F], mybir.dt.float16, kind="ExternalOutput")
    hnext = nc.dram_tensor("hnext", [PADN, F], mybir.dt.float16, kind="ExternalOutput")
    pooled = nc.dram_tensor("pooled", [B, F], mybir.dt.float32, kind="ExternalOutput")

    AF = mybir.ActivationFunctionType
    with tile.TileContext(nc) as tc:
        with (
            tc.tile_pool(name="cst", bufs=1) as cst,
            tc.tile_pool(name="sb", bufs=8) as pool,
            tc.tile_pool(name="psa", bufs=3, space="PSUM") as psa,
            tc.tile_pool(name="pst", bufs=2, space="PSUM") as pst,
            tc.tile_pool(name="psd", bufs=2, space="PSUM") as psd,
            tc.tile_pool(name="psp", bufs=1, space="PSUM") as psp,
            tc.tile_pool(name="dr", bufs=1, space="DRAM") as drp,
        ):
            # constants
            iota = cst.tile([128, 128], mybir.dt.float32)
            nc.gpsimd.iota(iota[:], pattern=[[1, 128]], base=0, channel_multiplier=0,
                           allow_small_or_imprecise_dtypes=True)
            pidx = cst.tile([128, 1], mybir.dt.float32)
            nc.gpsimd.iota(pidx[:], pattern=[[1, 1]], base=0, channel_multiplier=1,
                           allow_small_or_imprecise_dtypes=True)
            ident = cst.tile([128, 128], mybir.dt.float16)
            nc.vector.tensor_scalar(out=ident[:], in0=iota[:], scalar1=pidx[:, 0:1],
                                    scalar2=None, op0=mybir.AluOpType.is_equal)
            wsb = cst.tile([FK, F], mybir.dt.float16)
            nc.sync.dma_start(out=wsb[:], in_=wmat[:])

            # chunked AllGather: per-chunk staging buffers so each collective
            # only depends on its own chunk's writes and overlaps later windows
            NCH = 8
            bounds = [round(c * NWIN / NCH) for c in range(NCH + 1)]
            tshards = [drp.tile([(bounds[c + 1] - bounds[c]) * WIN, F],
                                mybir.dt.float16, tag=f"tshard{c}",
                                name=f"tshard{c}") for c in range(NCH)]
            chrows = [(bounds[c + 1] - bounds[c]) * WIN for c in range(NCH)]
            tfulls = [nc.dram_tensor(f"tfull{c}", [NCORES, chrows[c], F],
                                     mybir.dt.float16) for c in range(NCH)]
            pacc = psp.tile([128, F], mybir.dt.float32)

            for w in range(NWIN):
                it = pool.tile([128, TW], mybir.dt.int32, tag="it")
                nc.sync.dma_start(out=it[:], in_=sidx[w])
                fx = pool.tile([128, TW + 5], mybir.dt.float32, tag="fx")
                nc.sync.dma_start(out=fx[:], in_=faux[w])
                ht = pool.tile([128, F], mybir.dt.float16, tag="ht")
                nc.sync.dma_start(out=ht[:], in_=hst[w * WIN:(w + 1) * WIN, :])

                TWw = tw_list[w]
                g = pool.tile([128, TW * F], mybir.dt.float16, tag="g")
                for t in range(TWw):
                    nc.gpsimd.indirect_dma_start(
                        out=g[:, t * F:(t + 1) * F], out_offset=None, in_=table[:],
                        in_offset=bass.IndirectOffsetOnAxis(ap=it[:, t:t + 1], axis=0),
                    )
                oh = pool.tile([128, TW * 128], mybir.dt.float16, tag="oh")
                acc = psa.tile([128, F], mybir.dt.float32, tag="acc")
                for t in range(max(TWw, 1)):
                    nc.vector.tensor_scalar(
                        out=oh[:, t * 128:(t + 1) * 128], in0=iota[:],
                        scalar1=fx[:, t:t + 1], scalar2=None,
                        op0=mybir.AluOpType.is_equal)
                    nc.tensor.matmul(out=acc[:], lhsT=oh[:, t * 128:(t + 1) * 128],
                                     rhs=g[:, t * F:(t + 1) * F],
                                     start=(t == 0), stop=False)
                # self loop: acc += I^T @ (dinv ⊙ h)
                hs = pool.tile([128, F], mybir.dt.float16, tag="hs")
                nc.vector.tensor_scalar(out=hs[:, 0:D], in0=ht[:, 0:D],
                                        scalar1=fx[:, CD:CD + 1], scalar2=None,
                                        op0=mybir.AluOpType.mult)
                nc.vector.tensor_scalar(out=hs[:, D:F], in0=ht[:, D:F],
                                        scalar1=fx[:, CD + 1:CD + 2], scalar2=None,
                                        op0=mybir.AluOpType.mult)
                nc.tensor.matmul(out=acc[:], lhsT=ident[:], rhs=hs[:],
                                 start=False, stop=True)
                # evict with dst-side dinv scale: pre = dinv ⊙ acc
                pre = pool.tile([128, F], mybir.dt.float16, tag="pre")
                nc.scalar.activation(out=pre[:, 0:D], in_=acc[:, 0:D], func=AF.Copy,
                                     scale=fx[:, CD:CD + 1])
                nc.scalar.activation(out=pre[:, D:F], in_=acc[:, D:F], func=AF.Copy,
                                     scale=fx[:, CD + 1:CD + 2])
                # transpose, append ones row, dense W + bias, relu
                ptr = pst.tile([F, 128], mybir.dt.float16, tag="ptr")
                nc.tensor.transpose(ptr[:], pre[:], ident[:])
                preT = pool.tile([FK, 128], mybir.dt.float16, tag="preT")
                nc.vector.memset(preT[:], 1.0)
                nc.scalar.copy(out=preT[0:F, :], in_=ptr[:])
                dn = psd.tile([128, F], mybir.dt.float32, tag="dn")
                nc.tensor.matmul(out=dn[:], lhsT=preT[:], rhs=wsb[:],
                                 start=True, stop=True)
                hn = pool.tile([128, F], mybir.dt.float16, tag="hn")
                nc.scalar.activation(out=hn[:], in_=dn[:], func=AF.Relu)
                nc.sync.dma_start(out=hnext[w * WIN:(w + 1) * WIN, :], in_=hn[:])
                # next-layer send-table entries: [spro ⊙ hp | slig ⊙ hl]
                tb = pool.tile([128, F], mybir.dt.float16, tag="tb")
                nc.vector.tensor_scalar(out=tb[:, 0:D], in0=hn[:, 0:D],
                                        scalar1=fx[:, CD + 2:CD + 3], scalar2=None,
                                        op0=mybir.AluOpType.mult)
                nc.vector.tensor_scalar(out=tb[:, D:F], in0=hn[:, D:F],
                                        scalar1=fx[:, CD + 3:CD + 4], scalar2=None,
                                        op0=mybir.AluOpType.mult)
                ch = next(c for c in range(NCH) if bounds[c] <= w < bounds[c + 1])
                wl = w - bounds[ch]
                nc.sync.dma_start(out=tshards[ch][wl * WIN:(wl + 1) * WIN, :], in_=tb[:])
                # batch pooling of h (unscaled)
                ohb = pool.tile([128, 128], mybir.dt.float16, tag="ohb")
                nc.vector.tensor_scalar(out=ohb[:], in0=iota[:],
                                        scalar1=fx[:, CD + 4:CD + 5], scalar2=None,
                                        op0=mybir.AluOpType.is_equal)
                nc.tensor.matmul(out=pacc[:], lhsT=ohb[:], rhs=hn[:],
                                 start=(w == 0), stop=(w == NWIN - 1))
                if w + 1 == bounds[ch + 1]:
                    nc.gpsimd.collective_compute(
                        "AllGather", mybir.AluOpType.bypass,
                        replica_groups=[list(range(NCORES))],
                        ins=[tshards[ch][:].opt()],
                        outs=[tfulls[ch][:].opt()],
                    )
                    a = bounds[ch] * WIN
                    for k in range(NCORES):
                        nc.sync.dma_start(
                            out=tnext[k * PADN + a:k * PADN + a + chrows[ch], :],
                            in_=tfulls[ch][k])

            psb = pool.tile([128, F], mybir.dt.float32, tag="psb")
            nc.scalar.copy(out=psb[:], in_=pacc[:])
            nc.sync.dma_start(out=pooled[:], in_=psb[:])
    nc.compile()
    return nc


def _build_runner(nc, n_cores=NCORES, replicated=("table", "wmat"), rep_out=("tnext",)):
    import jax
    import concourse.mybir as mybir
    from jax.sharding import Mesh, PartitionSpec, NamedSharding
    from jax.experimental.shard_map import shard_map
    from concourse.bass2jax import _bass_exec_p, install_neuronx_cc_hook, partition_id_tensor

    install_neuronx_cc_hook()
    partition_name = nc.partition_id_tensor.name if nc.partition_id_tensor else None
    in_names, out_names, out_avals = [], [], []
    for alloc in nc.m.functions[0].allocations:
        if not isinstance(alloc, mybir.MemoryLocationSet):
            continue
        name = alloc.memorylocations[0].name
        if alloc.kind == "ExternalInput":
            if name != partition_name:
                in_names.append(name)
        elif alloc.kind == "ExternalOutput":
            shape = tuple(alloc.tensor_shape)
            np_dt = mybir.dt.np(alloc.dtype)
            out_avals.append(jax.core.ShapedArray(shape, np_dt))
            out_names.append(name)
    all_in_names = list(in_names) + list(out_names)
    if partition_name is not None:
        all_in_names.append(partition_name)

    def _body(*args):
        operands = list(args)
        if partition_name is not None:
            operands.append(partition_id_tensor())
        outs = _bass_exec_p.bind(
            *operands,
            out_avals=tuple(out_avals),
            in_names=tuple(all_in_names),
            out_names=tuple(out_names),
            lowering_input_output_aliases=(),
            sim_require_finite=True,
            sim_require_nnan=True,
            nc=nc,
        )
        return tuple(outs)

    devices = jax.devices()[:n_cores]
    mesh = Mesh(np.asarray(devices), ("core",))
    in_specs = tuple(
        PartitionSpec() if nm in replicated else PartitionSpec("core")
        for nm in in_names
    ) + tuple(
        PartitionSpec() if nm in rep_out else PartitionSpec("core")
        for nm in out_names
    )
    out_specs = tuple(
        PartitionSpec() if nm in rep_out else PartitionSpec("core")
        for nm in out_names
    )
    sharded = jax.jit(
        shard_map(_body, mesh=mesh, in_specs=in_specs, out_specs=out_specs,
                  check_rep=False),
        keep_unused=True,
    )
    replicate = jax.jit(
        shard_map(lambda t: jax.lax.all_gather(t, "core", axis=0, tiled=True),
                  mesh=mesh, in_specs=PartitionSpec("core"),
                  out_specs=PartitionSpec(), check_rep=False),
    )

    class R:
        input_names = in_names
        output_names = out_names
        avals = out_avals

        def __init__(self):
            self._zeros_dev = None
            self.mesh = mesh
            self.sharded = sharded
            self.rep_sharding = NamedSharding(mesh, PartitionSpec())
            self.shard_sharding = NamedSharding(mesh, PartitionSpec("core"))

        def put_replicated(self, arr):
            """Ship [M, ...] once (core-sharded over the tunnel), replicate
            on-device over NeuronLink."""
            import jax as _j
            r = replicate(_j.device_put(arr, self.shard_sharding))
            _j.block_until_ready(r)
            return r

        def _zeros(self):
            import jax as _j
            import jax.numpy as jnp
            if self._zeros_dev is None:
                zs = []
                for nm, av in zip(out_names, out_avals):
                    if nm in rep_out:
                        sh, shard = av.shape, self.rep_sharding
                    else:
                        sh, shard = (n_cores * av.shape[0], *av.shape[1:]), self.shard_sharding
                    zs.append(_j.jit(lambda s=sh, d=av.dtype: jnp.zeros(s, d),
                                     out_shardings=shard)())
                _j.block_until_ready(zs)
                self._zeros_dev = zs
            return self._zeros_dev

        def run_layers(self, base_args, tables, hst0, wmats, nlayer=NLAYER):
            """Run the per-layer NEFF nlayer times, chaining device-resident
            outputs to inputs. Returns dict of last layer's outputs."""
            import jax as _j
            import time as _t
            zeros = self._zeros()
            tbl, hcur = tables, hst0
            t0 = _t.perf_counter()
            outs = None
            for i in range(nlayer):
                args_by_name = {**base_args, "table": tbl, "hst": hcur,
                                "wmat": wmats[i]}
                args = [args_by_name[nm] for nm in in_names] + list(zeros)
                outs = sharded(*args)
                tbl = outs[out_names.index("tnext")]
                hcur = outs[out_names.index("hnext")]
            _j.block_until_ready(outs)
            self.last_exec_seconds = _t.perf_counter() - t0
            self.n_launches = nlayer
            return {nm: outs[i] for i, nm in enumerate(out_names)}

    return R()


def _prep_structure(edge_index, split, dinv_p, dinv_l, spro, slig, batch):
    """Balanced-window edge schedule: per core, dsts are dealt serpentine by
    edge count into NWIN windows (~E/core/NWIN edges each), the node order is
    permuted to (window, slot), and the send table lives in permuted order
    (NCORES*PADN rows). Returns sidx (permuted-table row ids), faux, and the
    per-core slot->original-node map."""
    src = edge_index[0].astype(np.int64)
    dst = edge_index[1].astype(np.int64)
    kd = dst // SHARD
    posmap = np.zeros(N, np.int64)       # original node id -> permuted table row
    nodeat = np.full((NCORES, PADN), -1, np.int64)  # permuted slot -> node id
    win_of = np.zeros(N, np.int64)
    slot_of = np.zeros(N, np.int64)
    cnt_all = np.bincount(dst, minlength=N)
    for k in range(NCORES):
        lo = k * SHARD
        cnt = cnt_all[lo:lo + SHARD]
        order = np.argsort(-cnt, kind="stable")      # dsts by degree desc
        i = np.arange(SHARD)
        rnd, pos = i // NWIN, i % NWIN
        w = np.where(rnd % 2 == 0, pos, NWIN - 1 - pos)   # serpentine deal
        p = rnd
        win_of[lo + order] = w
        slot_of[lo + order] = p
        nodeat[k, w * WIN + p] = lo + order
        posmap[lo + order] = k * PADN + w * WIN + p
    cores = []
    tw_req = 0
    for k in range(NCORES):
        lo, hi = k * SHARD, (k + 1) * SHARD
        m = (kd == k)
        s_k, d_k = src[m], dst[m]
        w_k, p_k = win_of[d_k], slot_of[d_k]
        order = np.argsort(w_k, kind="stable")
        s_k, w_k, p_k = s_k[order], w_k[order], p_k[order]
        counts = np.bincount(w_k, minlength=NWIN)
        tw_req = max(tw_req, int(np.max((counts + 127) // 128)))
        cores.append((s_k, p_k, counts))
    TW = max(tw_req, 1)
    tw_list = np.max(np.stack([(c[2] + 127) // 128 for c in cores]), axis=0)
    sidx_all = np.zeros((NCORES, NWIN, 128, TW), np.int32)
    faux_all = np.zeros((NCORES, NWIN, 128, TW + 5), np.float32)
    faux_all[..., 0:TW] = -1.0
    for k, (s_k, p_k, counts) in enumerate(cores):
        starts = np.concatenate([[0], np.cumsum(counts)])
        for w in range(NWIN):
            a, b = starts[w], starts[w + 1]
            n = b - a
            if n:
                sl = np.arange(n)
                tt, pp = sl // 128, sl % 128
                sidx_all[k, w, pp, tt] = posmap[s_k[a:b]]
                faux_all[k, w, pp, tt] = p_k[a:b].astype(np.float32)
        gn = nodeat[k]
        valid = gn >= 0
        gc = np.maximum(gn, 0)
        fa = faux_all[k].reshape(PADN, TW + 5)
        fa[:, TW + 0] = np.where(valid, dinv_p[gc], 0.0)
        fa[:, TW + 1] = np.where(valid, dinv_l[gc], 0.0)
        fa[:, TW + 2] = np.where(valid, spro[gc], 0.0)
        fa[:, TW + 3] = np.where(valid, slig[gc], 0.0)
        fa[:, TW + 4] = np.where(valid, batch[gc].astype(np.float32), -1.0)
    return TW, tuple(int(v) for v in tw_list), sidx_all, faux_all, nodeat


def _tail(inputs, pro, lig):
    seq = np.zeros((T, B, D), np.float32)
    seq[0] = lig
    seq[1] = pro

    def lstm(wih, whh, bias, reverse):
        hs = np.zeros((T, B, D), np.float32)
        hh = np.zeros((B, D), np.float32)
        c = np.zeros((B, D), np.float32)
        order = range(T - 1, -1, -1) if reverse else range(T)
        sig = lambda z: 1.0 / (1.0 + np.exp(-z))
        for t in order:
            g = seq[t] @ wih.T + hh @ whh.T + bias
            i_, f_, g_, o_ = g[:, :35], g[:, 35:70], g[:, 70:105], g[:, 105:]
            c = sig(f_) * c + sig(i_) * np.tanh(g_)
            hh = sig(o_) * np.tanh(c)
            hs[t] = hh
        return hs

    hf = lstm(inputs["wif"], inputs["whf"], inputs["bif"] + inputs["bhf"], False)
    hb = lstm(inputs["wib"], inputs["whb"], inputs["bib"] + inputs["bhb"], True)
    out = np.concatenate([hf, hb], axis=-1).transpose(1, 0, 2)
    q = out @ inputs["Wq"].T + inputs["bq"]
    k = out @ inputs["Wk"].T + inputs["bk"]
    v = out @ inputs["Wv"].T + inputs["bv"]
    scores = np.einsum('btd,bsd->bts', q, k) / np.sqrt(np.float32(70))
    scores = np.where(_mask_mat(), scores, np.float32(-1e9))
    e = np.exp(scores - scores.max(-1, keepdims=True))
    att = e / e.sum(-1, keepdims=True)
    ctx = att @ v
    ctx = ctx @ inputs["Wo"].T + inputs["bo"]
    y = ctx.reshape(B, -1) @ inputs["W1"].T + inputs["b1"]
    mu = y.mean(0)
    var = ((y - mu) ** 2).mean(0)
    y = (y - mu) / np.sqrt(var + 1e-5) * inputs["gamma"] + inputs["beta"]
    y = y * np.tanh(np.log1p(np.exp(-np.abs(y))) + np.maximum(y, 0))
    return (y @ inputs["W2"].T + inputs["b2"]).reshape(-1).astype(np.float32)


def kernel(**inputs):
    inputs = {k: np.asarray(v) for k, v in inputs.items()}
    x = inputs["x"].astype(np.float32)
    edge_index = inputs["edge_index"]
    split = inputs["split"].astype(np.int64)
    batch = inputs["batch"].astype(np.int64)
    Wp, bp = inputs["Wp"].astype(np.float32), inputs["bp"].astype(np.float32)
    Wl, bl = inputs["Wl"].astype(np.float32), inputs["bl"].astype(np.float32)

    src = edge_index[0].astype(np.int64)
    dst = edge_index[1].astype(np.int64)
    wpro = split[src] == 1
    deg_p = np.bincount(dst[wpro], minlength=N) + 1.0
    deg_l = np.bincount(dst[~wpro], minlength=N) + 1.0
    dinv_p = (1.0 / np.sqrt(deg_p)).astype(np.float32)
    dinv_l = (1.0 / np.sqrt(deg_l)).astype(np.float32)
    spro = np.where(split == 1, dinv_p, 0).astype(np.float32)
    slig = np.where(split == 0, dinv_l, 0).astype(np.float32)

    import jax
    key = (edge_index.shape, int(edge_index[:, ::9973].astype(np.int64).sum()))
    if _CACHE.get("key") != key:
        TW, tw_list, sidx, faux, nodeat = _prep_structure(edge_index, split,
                                                  dinv_p, dinv_l, spro, slig, batch)
        _CACHE["nodeat"] = nodeat
        if _CACHE.get("prog_key") != (TW, tw_list):
            nc = _build_device_program(TW, list(tw_list))
            _CACHE["runner"] = _build_runner(nc)
            _CACHE["tw"] = TW
            _CACHE["prog_key"] = (TW, tw_list)
        assert TW <= _CACHE["tw"], "edge distribution needs more tiles/window"
        TWc = _CACHE["tw"]
        if TW < TWc:  # pad schedule to the compiled TW
            sidx2 = np.zeros((NCORES, NWIN, 128, TWc), np.int32)
            faux2 = np.zeros((NCORES, NWIN, 128, TWc + 5), np.float32)
            faux2[..., 0:TWc] = -1.0
            sidx2[..., :TW] = sidx
            faux2[..., :TW] = faux[..., :TW]
            faux2[..., TWc:] = faux[..., TW:]
            sidx, faux = sidx2, faux2
        r = _CACHE["runner"]
        _CACHE["sidx_dev"] = jax.device_put(
            sidx.reshape(NCORES * NWIN, 128, _CACHE["tw"]), r.shard_sharding)
        _CACHE["faux_dev"] = jax.device_put(
            faux.reshape(NCORES * NWIN, 128, _CACHE["tw"] + 5), r.shard_sharding)
        jax.block_until_ready([_CACHE["sidx_dev"], _CACHE["faux_dev"]])
        _CACHE["key"] = key
    runner = _CACHE["runner"]

    # per-layer dense weights: block-diag(Wp, Wl) + bias row, fp16 (tiny:
    # direct replicated device_put)
    import jax as _j
    wmats = []
    for i in range(NLAYER):
        w71 = np.zeros((FK, F), np.float32)
        w71[0:D, 0:D] = Wp[i]
        w71[D:F, D:F] = Wl[i]
        w71[F, 0:D] = bp[i]
        w71[F, D:F] = bl[i]
        wmats.append(_j.device_put(w71.astype(np.float16), runner.rep_sharding))
    _j.block_until_ready(wmats)

    # initial table and h-state, in permuted (window, slot) node order
    nodeat = _CACHE["nodeat"]
    valid = (nodeat >= 0)[..., None]
    gc = np.maximum(nodeat, 0)
    xp = np.where(valid, x[gc.reshape(-1)].reshape(NCORES, PADN, D), 0)
    table0 = np.concatenate([
        np.where(valid, spro[gc.reshape(-1)].reshape(NCORES, PADN)[..., None], 0) * xp,
        np.where(valid, slig[gc.reshape(-1)].reshape(NCORES, PADN)[..., None], 0) * xp,
    ], axis=2).astype(np.float16).reshape(NCORES * PADN, F)
    tbl_dev = runner.put_replicated(table0)
    h0p = np.concatenate([xp, xp], axis=2).astype(np.float16)
    hst_dev = _j.device_put(h0p.reshape(NCORES * PADN, F), runner.shard_sharding)
    _j.block_until_ready(hst_dev)

    base_args = {"sidx": _CACHE["sidx_dev"], "faux": _CACHE["faux_dev"]}
    res = runner.run_layers(base_args, tbl_dev, hst_dev, wmats)
    kernel.last_device_seconds = runner.last_exec_seconds
    kernel.last_n_launches = runner.n_launches

    pooled = np.asarray(res["pooled"]).reshape(NCORES, B, F).sum(0)
    pro, lig = pooled[:, :D].astype(np.float32), pooled[:, D:].astype(np.float32)

    _CACHE["bench"] = (base_args, tbl_dev, hst_dev, wmats)
    return _tail(inputs, pro, lig)


def bench_chain(nlayer):
    """Wall time of an nlayer chain on device-resident inputs (timing aid for
    test.py's slope-based device-time estimate). Requires a prior kernel()."""
    import time as _t
    runner = _CACHE["runner"]
    base_args, tbl_dev, hst_dev, wmats = _CACHE["bench"]
    wm = (wmats * ((nlayer + NLAYER - 1) // NLAYER))[:nlayer]
    t0 = _t.perf_counter()
    runner.run_layers(base_args, tbl_dev, hst_dev, wm, nlayer=nlayer)
    return _t.perf_counter() - t0


# revision 21
# speedup vs baseline: 1.0323x; 1.0323x over previous
"""Trainium2 Bass kernel for nn_ASD_72069551227061 (GNN message passing).

Strategy (8 NeuronCores, dst-sharded graph parallelism, fully on-device GCN):
  Each of the 5 GCN layers runs as ONE NEFF launch per layer; the NEFF does,
  per core (50k-dst shard, 391 windows of 128 dsts):
    - indirect-DMA gather of the 6.4M/8 edge messages from a replicated
      [N,70] fp16 send table (pro|lig packed halves),
    - segment-sum by destination via one-hot S-matrix matmuls in PSUM,
      where the one-hots are GENERATED ON DEVICE (iota + is_equal) instead
      of streamed from HBM (kills the 218MB/layer smat traffic of v1),
    - the self-loop term via a diag-scaled identity matmul,
    - the dst-side dinv scaling on PSUM eviction (per-partition scale),
    - the dense 35x35 GCN weight matmul + bias + ReLU on the PE (bias via a
      ones-row in the transposed activations),
    - the next layer's send table (spro/slig-scaled h), re-replicated across
      the 8 cores by a gpsimd AllGather INSIDE the NEFF,
    - batch pooling (one-hot over batch ids) accumulated in PSUM.
  Host work between launches is zero: layer k+1's inputs are layer k's
  device-resident outputs. Host does degrees/scales/table0 prep (untimed)
  and the tiny [128,...] BiLSTM/attention/MLP tail in fp32.
"""
import sys
sys.path.insert(0, "/opt/trn_rl_repo")

import numpy as np
import ml_dtypes

N, E, B, D, T, SEQ = 400_000, 6_400_000, 128, 35, 140, 2
F = 2 * D                    # 70: packed [pro | lig] feature row
FK = F + 1                   # 71: + ones row for bias
NCORES = 8
SHARD = N // NCORES          # 50_000 dsts per core
WIN = 128                    # dsts per PSUM window
NWIN = (SHARD + WIN - 1) // WIN   # 391 windows
PADN = NWIN * WIN            # 50_048 padded dst rows per core
NLAYER = 5

_CACHE = {}


def _mask_mat():
    m = np.eye(T, dtype=bool)
    m[SEQ:, :] = False
    m[:, SEQ:] = False
    m[:, SEQ - 1] = True
    m[SEQ - 1, :] = True
    m[SEQ - 1, SEQ - 1] = False
    return m


def _build_device_program(n_tiles_per_win, tw_list=None):
    import concourse.bass as bass
    import concourse.bacc as bacc
    import concourse.mybir as mybir
    import concourse.tile as tile

    TW = n_tiles_per_win
    if tw_list is None:
        tw_list = [TW] * NWIN
    CD = TW  # faux col offsets: dloc cols [0:TW], then scales
    nc = bacc.Bacc("TRN2", target_bir_lowering=False, debug=False, num_devices=NCORES)
    NP = NCORES * PADN
    table = nc.dram_tensor("table", [NP, F], mybir.dt.float16, kind="ExternalInput")
    hst = nc.dram_tensor("hst", [PADN, F], mybir.dt.float16, kind="ExternalInput")
    sidx = nc.dram_tensor("sidx", [NWIN, 128, TW], mybir.dt.int32, kind="ExternalInput")
    faux = nc.dram_tensor("faux", [NWIN, 128, TW + 5], mybir.dt.float32, kind="ExternalInput")
    wmat = nc.dram_tensor("wmat", [FK, F], mybir.dt.float16, kind="ExternalInput")
    tnext = nc.dram_tensor("tnext", [NP, F], mybir.dt.float16, kind="ExternalOutput")
    hnext = nc.dram_tensor("hnext", [PADN, F], mybir.dt.float16, kind="ExternalOutput")
    pooled = nc.dram_tensor("pooled", [B, F], mybir.dt.float32, kind="ExternalOutput")

    AF = mybir.ActivationFunctionType
    with tile.TileContext(nc) as tc:
        with (
            tc.tile_pool(name="cst", bufs=1) as cst,
            tc.tile_pool(name="sb", bufs=8) as pool,
            tc.tile_pool(name="psa", bufs=2, space="PSUM") as psa,
            tc.tile_pool(name="pst", bufs=2, space="PSUM") as pst,
            tc.tile_pool(name="psd", bufs=2, space="PSUM") as psd,
            tc.tile_pool(name="psp", bufs=1, space="PSUM") as psp,
            tc.tile_pool(name="dr", bufs=1, space="DRAM") as drp,
        ):
            # constants
            iota = cst.tile([128, 128], mybir.dt.float32)
            nc.gpsimd.iota(iota[:], pattern=[[1, 128]], base=0, channel_multiplier=0,
                           allow_small_or_imprecise_dtypes=True)
            pidx = cst.tile([128, 1], mybir.dt.float32)
            nc.gpsimd.iota(pidx[:], pattern=[[1, 1]], base=0, channel_multiplier=1,
                           allow_small_or_imprecise_dtypes=True)
            ident = cst.tile([128, 128], mybir.dt.float16)
            nc.vector.tensor_scalar(out=ident[:], in0=iota[:], scalar1=pidx[:, 0:1],
                                    scalar2=None, op0=mybir.AluOpType.is_equal)
            wsb = cst.tile([FK, F], mybir.dt.float16)
            nc.sync.dma_start(out=wsb[:], in_=wmat[:])

            # chunked AllGather: per-chunk staging buffers so each collective
            # only depends on its own chunk's writes and overlaps later windows
            NCH = 8
            bounds = [round(c * NWIN / NCH) for c in range(NCH + 1)]
            tshards = [drp.tile([(bounds[c + 1] - bounds[c]) * WIN, F],
                                mybir.dt.float16, tag=f"tshard{c}",
                                name=f"tshard{c}") for c in range(NCH)]
            chrows = [(bounds[c + 1] - bounds[c]) * WIN for c in range(NCH)]
            tfulls = [nc.dram_tensor(f"tfull{c}", [NCORES, chrows[c], F],
                                     mybir.dt.float16) for c in range(NCH)]
            pacc = psp.tile([128, F], mybir.dt.float32)

            for w in range(NWIN):
                it = pool.tile([128, TW], mybir.dt.int32, tag="it")
                nc.sync.dma_start(out=it[:], in_=sidx[w])
                fx = pool.tile([128, TW + 5], mybir.dt.float32, tag="fx")
                nc.sync.dma_start(out=fx[:], in_=faux[w])
                ht = pool.tile([128, F], mybir.dt.float16, tag="ht")
                nc.sync.dma_start(out=ht[:], in_=hst[w * WIN:(w + 1) * WIN, :])

                TWw = tw_list[w]
                g = pool.tile([128, TW * F], mybir.dt.float16, tag="g")
                for t in range(TWw):
                    nc.gpsimd.indirect_dma_start(
                        out=g[:, t * F:(t + 1) * F], out_offset=None, in_=table[:],
                        in_offset=bass.IndirectOffsetOnAxis(ap=it[:, t:t + 1], axis=0),
                    )
                oh = pool.tile([128, TW * 128], mybir.dt.float16, tag="oh")
                acc = psa.tile([128, F], mybir.dt.float32, tag="acc")
                for t in range(max(TWw, 1)):
                    nc.vector.tensor_scalar(
                        out=oh[:, t * 128:(t + 1) * 128], in0=iota[:],
                        scalar1=fx[:, t:t + 1], scalar2=None,
                        op0=mybir.AluOpType.is_equal)
                    nc.tensor.matmul(out=acc[:], lhsT=oh[:, t * 128:(t + 1) * 128],
                                     rhs=g[:, t * F:(t + 1) * F],
                                     start=(t == 0), stop=False)
                # self loop: acc += I^T @ (dinv ⊙ h)
                hs = pool.tile([128, F], mybir.dt.float16, tag="hs")
                nc.vector.tensor_scalar(out=hs[:, 0:D], in0=ht[:, 0:D],
                                        scalar1=fx[:, CD:CD + 1], scalar2=None,
                                        op0=mybir.AluOpType.mult)
                nc.vector.tensor_scalar(out=hs[:, D:F], in0=ht[:, D:F],
                                        scalar1=fx[:, CD + 1:CD + 2], scalar2=None,
                                        op0=mybir.AluOpType.mult)
                nc.tensor.matmul(out=acc[:], lhsT=ident[:], rhs=hs[:],
                                 start=False, stop=True)
                # evict with dst-side dinv scale: pre = dinv ⊙ acc
                pre = pool.tile([128, F], mybir.dt.float16, tag="pre")
                nc.scalar.activation(out=pre[:, 0:D], in_=acc[:, 0:D], func=AF.Copy,
                                     scale=fx[:, CD:CD + 1])
                nc.scalar.activation(out=pre[:, D:F], in_=acc[:, D:F], func=AF.Copy,
                                     scale=fx[:, CD + 1:CD + 2])
                # transpose, append ones row, dense W + bias, relu
                ptr = pst.tile([F, 128], mybir.dt.float16, tag="ptr")
                nc.tensor.transpose(ptr[:], pre[:], ident[:])
                preT = pool.tile([FK, 128], mybir.dt.float16, tag="preT")
                nc.vector.memset(preT[:], 1.0)
                nc.scalar.copy(out=preT[0:F, :], in_=ptr[:])
                dn = psd.tile([128, F], mybir.dt.float32, tag="dn")
                nc.tensor.matmul(out=dn[:], lhsT=preT[:], rhs=wsb[:],
                                 start=True, stop=True)
                hn = pool.tile([128, F], mybir.dt.float16, tag="hn")
                nc.scalar.activation(out=hn[:], in_=dn[:], func=AF.Relu)
                nc.sync.dma_start(out=hnext[w * WIN:(w + 1) * WIN, :], in_=hn[:])
                # next-layer send-table entries: [spro ⊙ hp | slig ⊙ hl]
                tb = pool.tile([128, F], mybir.dt.float16, tag="tb")
                nc.vector.tensor_scalar(out=tb[:, 0:D], in0=hn[:, 0:D],
                                        scalar1=fx[:, CD + 2:CD + 3], scalar2=None,
                                        op0=mybir.AluOpType.mult)
                nc.vector.tensor_scalar(out=tb[:, D:F], in0=hn[:, D:F],
                                        scalar1=fx[:, CD + 3:CD + 4], scalar2=None,
                                        op0=mybir.AluOpType.mult)
                ch = next(c for c in range(NCH) if bounds[c] <= w < bounds[c + 1])
                wl = w - bounds[ch]
                nc.sync.dma_start(out=tshards[ch][wl * WIN:(wl + 1) * WIN, :], in_=tb[:])
                # batch pooling of h (unscaled)
                ohb = pool.tile([128, 128], mybir.dt.float16, tag="ohb")
                nc.vector.tensor_scalar(out=ohb[:], in0=iota[:],
                                        scalar1=fx[:, CD + 4:CD + 5], scalar2=None,
                                        op0=mybir.AluOpType.is_equal)
                nc.tensor.matmul(out=pacc[:], lhsT=ohb[:], rhs=hn[:],
                                 start=(w == 0), stop=(w == NWIN - 1))
                if w + 1 == bounds[ch + 1]:
                    nc.gpsimd.collective_compute(
                        "AllGather", mybir.AluOpType.bypass,
                        replica_groups=[list(range(NCORES))],
                        ins=[tshards[ch][:].opt()],
                        outs=[tfulls[ch][:].opt()],
                    )

            psb = pool.tile([128, F], mybir.dt.float32, tag="psb")
            nc.scalar.copy(out=psb[:], in_=pacc[:])
            nc.sync.dma_start(out=pooled[:], in_=psb[:])
            for c in range(NCH):
                a = bounds[c] * WIN
                for k in range(NCORES):
                    nc.sync.dma_start(
                        out=tnext[k * PADN + a:k * PADN + a + chrows[c], :],
                        in_=tfulls[c][k])
    nc.compile()
    return nc


def _build_runner(nc, n_cores=NCORES, replicated=("table", "wmat"), rep_out=("tnext",)):
    import jax
    import concourse.mybir as mybir
    from jax.sharding import Mesh, PartitionSpec, NamedSharding
    from jax.experimental.shard_map import shard_map
    from concourse.bass2jax import _bass_exec_p, install_neuronx_cc_hook, partition_id_tensor

    install_neuronx_cc_hook()
    partition_name = nc.partition_id_tensor.name if nc.partition_id_tensor else None
    in_names, out_names, out_avals = [], [], []
    for alloc in nc.m.functions[0].allocations:
        if not isinstance(alloc, mybir.MemoryLocationSet):
            continue
        name = alloc.memorylocations[0].name
        if alloc.kind == "ExternalInput":
            if name != partition_name:
                in_names.append(name)
        elif alloc.kind == "ExternalOutput":
            shape = tuple(alloc.tensor_shape)
            np_dt = mybir.dt.np(alloc.dtype)
            out_avals.append(jax.core.ShapedArray(shape, np_dt))
            out_names.append(name)
    all_in_names = list(in_names) + list(out_names)
    if partition_name is not None:
        all_in_names.append(partition_name)

    def _body(*args):
        operands = list(args)
        if partition_name is not None:
            operands.append(partition_id_tensor())
        outs = _bass_exec_p.bind(
            *operands,
            out_avals=tuple(out_avals),
            in_names=tuple(all_in_names),
            out_names=tuple(out_names),
            lowering_input_output_aliases=(),
            sim_require_finite=True,
            sim_require_nnan=True,
            nc=nc,
        )
        return tuple(outs)

    devices = jax.devices()[:n_cores]
    mesh = Mesh(np.asarray(devices), ("core",))
    in_specs = tuple(
        PartitionSpec() if nm in replicated else PartitionSpec("core")
        for nm in in_names
    ) + tuple(
        PartitionSpec() if nm in rep_out else PartitionSpec("core")
        for nm in out_names
    )
    out_specs = tuple(
        PartitionSpec() if nm in rep_out else PartitionSpec("core")
        for nm in out_names
    )
    sharded = jax.jit(
        shard_map(_body, mesh=mesh, in_specs=in_specs, out_specs=out_specs,
                  check_rep=False),
        keep_unused=True,
    )
    replicate = jax.jit(
        shard_map(lambda t: jax.lax.all_gather(t, "core", axis=0, tiled=True),
                  mesh=mesh, in_specs=PartitionSpec("core"),
                  out_specs=PartitionSpec(), check_rep=False),
    )

    class R:
        input_names = in_names
        output_names = out_names
        avals = out_avals

        def __init__(self):
            self._zeros_dev = None
            self.mesh = mesh
            self.sharded = sharded
            self.rep_sharding = NamedSharding(mesh, PartitionSpec())
            self.shard_sharding = NamedSharding(mesh, PartitionSpec("core"))

        def put_replicated(self, arr):
            """Ship [M, ...] once (core-sharded over the tunnel), replicate
            on-device over NeuronLink."""
            import jax as _j
            r = replicate(_j.device_put(arr, self.shard_sharding))
            _j.block_until_ready(r)
            return r

        def _zeros(self):
            import jax as _j
            import jax.numpy as jnp
            if self._zeros_dev is None:
                zs = []
                for nm, av in zip(out_names, out_avals):
                    if nm in rep_out:
                        sh, shard = av.shape, self.rep_sharding
                    else:
                        sh, shard = (n_cores * av.shape[0], *av.shape[1:]), self.shard_sharding
                    zs.append(_j.jit(lambda s=sh, d=av.dtype: jnp.zeros(s, d),
                                     out_shardings=shard)())
                _j.block_until_ready(zs)
                self._zeros_dev = zs
            return self._zeros_dev

        def run_layers(self, base_args, tables, hst0, wmats, nlayer=NLAYER):
            """Run the per-layer NEFF nlayer times, chaining device-resident
            outputs to inputs. Returns dict of last layer's outputs."""
            import jax as _j
            import time as _t
            zeros = self._zeros()
            tbl, hcur = tables, hst0
            t0 = _t.perf_counter()
            outs = None
            for i in range(nlayer):
                args_by_name = {**base_args, "table": tbl, "hst": hcur,
                                "wmat": wmats[i]}
                args = [args_by_name[nm] for nm in in_names] + list(zeros)
                outs = sharded(*args)
                tbl = outs[out_names.index("tnext")]
                hcur = outs[out_names.index("hnext")]
            _j.block_until_ready(outs)
            self.last_exec_seconds = _t.perf_counter() - t0
            self.n_launches = nlayer
            return {nm: outs[i] for i, nm in enumerate(out_names)}

    return R()


def _prep_structure(edge_index, split, dinv_p, dinv_l, spro, slig, batch):
    """Balanced-window edge schedule: per core, dsts are dealt serpentine by
    edge count into NWIN windows (~E/core/NWIN edges each), the node order is
    permuted to (window, slot), and the send table lives in permuted order
    (NCORES*PADN rows). Returns sidx (permuted-table row ids), faux, and the
    per-core slot->original-node map."""
    src = edge_index[0].astype(np.int64)
    dst = edge_index[1].astype(np.int64)
    kd = dst // SHARD
    posmap = np.zeros(N, np.int64)       # original node id -> permuted table row
    nodeat = np.full((NCORES, PADN), -1, np.int64)  # permuted slot -> node id
    win_of = np.zeros(N, np.int64)
    slot_of = np.zeros(N, np.int64)
    cnt_all = np.bincount(dst, minlength=N)
    for k in range(NCORES):
        lo = k * SHARD
        cnt = cnt_all[lo:lo + SHARD]
        order = np.argsort(-cnt, kind="stable")      # dsts by degree desc
        i = np.arange(SHARD)
        rnd, pos = i // NWIN, i % NWIN
        w = np.where(rnd % 2 == 0, pos, NWIN - 1 - pos)   # serpentine deal
        p = rnd
        win_of[lo + order] = w
        slot_of[lo + order] = p
        nodeat[k, w * WIN + p] = lo + order
        posmap[lo + order] = k * PADN + w * WIN + p
    cores = []
    tw_req = 0
    for k in range(NCORES):
        lo, hi = k * SHARD, (k + 1) * SHARD
        m = (kd == k)
        s_k, d_k = src[m], dst[m]
        w_k, p_k = win_of[d_k], slot_of[d_k]
        order = np.argsort(w_k, kind="stable")
        s_k, w_k, p_k = s_k[order], w_k[order], p_k[order]
        counts = np.bincount(w_k, minlength=NWIN)
        tw_req = max(tw_req, int(np.max((counts + 127) // 128)))
        cores.append((s_k, p_k, counts))
    TW = max(tw_req, 1)
    tw_list = np.max(np.stack([(c[2] + 127) // 128 for c in cores]), axis=0)
    sidx_all = np.zeros((NCORES, NWIN, 128, TW), np.int32)
    faux_all = np.zeros((NCORES, NWIN, 128, TW + 5), np.float32)
    faux_all[..., 0:TW] = -1.0
    for k, (s_k, p_k, counts) in enumerate(cores):
        starts = np.concatenate([[0], np.cumsum(counts)])
        for w in range(NWIN):
            a, b = starts[w], starts[w + 1]
            n = b - a
            if n:
                sl = np.arange(n)
                tt, pp = sl // 128, sl % 128
                sidx_all[k, w, pp, tt] = posmap[s_k[a:b]]
                faux_all[k, w, pp, tt] = p_k[a:b].astype(np.float32)
        gn = nodeat[k]
        valid = gn >= 0
        gc = np.maximum(gn, 0)
        fa = faux_all[k].reshape(PADN, TW + 5)
        fa[:, TW + 0] = np.where(valid, dinv_p[gc], 0.0)
        fa[:, TW + 1] = np.where(valid, dinv_l[gc], 0.0)
        fa[:, TW + 2] = np.where(valid, spro[gc], 0.0)
        fa[:, TW + 3] = np.where(valid, slig[gc], 0.0)
        fa[:, TW + 4] = np.where(valid, batch[gc].astype(np.float32), -1.0)
    return TW, tuple(int(v) for v in tw_list), sidx_all, faux_all, nodeat


def _tail(inputs, pro, lig):
    seq = np.zeros((T, B, D), np.float32)
    seq[0] = lig
    seq[1] = pro

    def lstm(wih, whh, bias, reverse):
        hs = np.zeros((T, B, D), np.float32)
        hh = np.zeros((B, D), np.float32)
        c = np.zeros((B, D), np.float32)
        order = range(T - 1, -1, -1) if reverse else range(T)
        sig = lambda z: 1.0 / (1.0 + np.exp(-z))
        for t in order:
            g = seq[t] @ wih.T + hh @ whh.T + bias
            i_, f_, g_, o_ = g[:, :35], g[:, 35:70], g[:, 70:105], g[:, 105:]
            c = sig(f_) * c + sig(i_) * np.tanh(g_)
            hh = sig(o_) * np.tanh(c)
            hs[t] = hh
        return hs

    hf = lstm(inputs["wif"], inputs["whf"], inputs["bif"] + inputs["bhf"], False)
    hb = lstm(inputs["wib"], inputs["whb"], inputs["bib"] + inputs["bhb"], True)
    out = np.concatenate([hf, hb], axis=-1).transpose(1, 0, 2)
    q = out @ inputs["Wq"].T + inputs["bq"]
    k = out @ inputs["Wk"].T + inputs["bk"]
    v = out @ inputs["Wv"].T + inputs["bv"]
    scores = np.einsum('btd,bsd->bts', q, k) / np.sqrt(np.float32(70))
    scores = np.where(_mask_mat(), scores, np.float32(-1e9))
    e = np.exp(scores - scores.max(-1, keepdims=True))
    att = e / e.sum(-1, keepdims=True)
    ctx = att @ v
    ctx = ctx @ inputs["Wo"].T + inputs["bo"]
    y = ctx.reshape(B, -1) @ inputs["W1"].T + inputs["b1"]
    mu = y.mean(0)
    var = ((y - mu) ** 2).mean(0)
    y = (y - mu) / np.sqrt(var + 1e-5) * inputs["gamma"] + inputs["beta"]
    y = y * np.tanh(np.log1p(np.exp(-np.abs(y))) + np.maximum(y, 0))
    return (y @ inputs["W2"].T + inputs["b2"]).reshape(-1).astype(np.float32)


def kernel(**inputs):
    inputs = {k: np.asarray(v) for k, v in inputs.items()}
    x = inputs["x"].astype(np.float32)
    edge_index = inputs["edge_index"]
    split = inputs["split"].astype(np.int64)
    batch = inputs["batch"].astype(np.int64)
    Wp, bp = inputs["Wp"].astype(np.float32), inputs["bp"].astype(np.float32)
    Wl, bl = inputs["Wl"].astype(np.float32), inputs["bl"].astype(np.float32)

    src = edge_index[0].astype(np.int64)
    dst = edge_index[1].astype(np.int64)
    wpro = split[src] == 1
    deg_p = np.bincount(dst[wpro], minlength=N) + 1.0
    deg_l = np.bincount(dst[~wpro], minlength=N) + 1.0
    dinv_p = (1.0 / np.sqrt(deg_p)).astype(np.float32)
    dinv_l = (1.0 / np.sqrt(deg_l)).astype(np.float32)
    spro = np.where(split == 1, dinv_p, 0).astype(np.float32)
    slig = np.where(split == 0, dinv_l, 0).astype(np.float32)

    import jax
    key = (edge_index.shape, int(edge_index[:, ::9973].astype(np.int64).sum()))
    if _CACHE.get("key") != key:
        TW, tw_list, sidx, faux, nodeat = _prep_structure(edge_index, split,
                                                  dinv_p, dinv_l, spro, slig, batch)
        _CACHE["nodeat"] = nodeat
        if _CACHE.get("prog_key") != (TW, tw_list):
            nc = _build_device_program(TW, list(tw_list))
            _CACHE["runner"] = _build_runner(nc)
            _CACHE["tw"] = TW
            _CACHE["prog_key"] = (TW, tw_list)
        assert TW <= _CACHE["tw"], "edge distribution needs more tiles/window"
        TWc = _CACHE["tw"]
        if TW < TWc:  # pad schedule to the compiled TW
            sidx2 = np.zeros((NCORES, NWIN, 128, TWc), np.int32)
            faux2 = np.zeros((NCORES, NWIN, 128, TWc + 5), np.float32)
            faux2[..., 0:TWc] = -1.0
            sidx2[..., :TW] = sidx
            faux2[..., :TW] = faux[..., :TW]
            faux2[..., TWc:] = faux[..., TW:]
            sidx, faux = sidx2, faux2
        r = _CACHE["runner"]
        _CACHE["sidx_dev"] = jax.device_put(
            sidx.reshape(NCORES * NWIN, 128, _CACHE["tw"]), r.shard_sharding)
        _CACHE["faux_dev"] = jax.device_put(
            faux.reshape(NCORES * NWIN, 128, _CACHE["tw"] + 5), r.shard_sharding)
        jax.block_until_ready([_CACHE["sidx_dev"], _CACHE["faux_dev"]])
        _CACHE["key"] = key
    runner = _CACHE["runner"]

    # per-layer dense weights: block-diag(Wp, Wl) + bias row, fp16 (tiny:
    # direct replicated device_put)
    import jax as _j
    wmats = []
    for i in range(NLAYER):
        w71 = np.zeros((FK, F), np.float32)
        w71[0:D, 0:D] = Wp[i]
        w71[D:F, D:F] = Wl[i]
        w71[F, 0:D] = bp[i]
        w71[F, D:F] = bl[i]
        wmats.append(_j.device_put(w71.astype(np.float16), runner.rep_sharding))
    _j.block_until_ready(wmats)

    # initial table and h-state, in permuted (window, slot) node order
    nodeat = _CACHE["nodeat"]
    valid = (nodeat >= 0)[..., None]
    gc = np.maximum(nodeat, 0)
    xp = np.where(valid, x[gc.reshape(-1)].reshape(NCORES, PADN, D), 0)
    table0 = np.concatenate([
        np.where(valid, spro[gc.reshape(-1)].reshape(NCORES, PADN)[..., None], 0) * xp,
        np.where(valid, slig[gc.reshape(-1)].reshape(NCORES, PADN)[..., None], 0) * xp,
    ], axis=2).astype(np.float16).reshape(NCORES * PADN, F)
    tbl_dev = runner.put_replicated(table0)
    h0p = np.concatenate([xp, xp], axis=2).astype(np.float16)
    hst_dev = _j.device_put(h0p.reshape(NCORES * PADN, F), runner.shard_sharding)
    _j.block_until_ready(hst_dev)

    base_args = {"sidx": _CACHE["sidx_dev"], "faux": _CACHE["faux_dev"]}
    res = runner.run_layers(base_args, tbl_dev, hst_dev, wmats)
    kernel.last_device_seconds = runner.last_exec_seconds
    kernel.last_n_launches = runner.n_launches

    pooled = np.asarray(res["pooled"]).reshape(NCORES, B, F).sum(0)
    pro, lig = pooled[:, :D].astype(np.float32), pooled[:, D:].astype(np.float32)

    _CACHE["bench"] = (base_args, tbl_dev, hst_dev, wmats)
    return _tail(inputs, pro, lig)


def bench_chain(nlayer):
    """Wall time of an nlayer chain on device-resident inputs (timing aid for
    test.py's slope-based device-time estimate). Requires a prior kernel()."""
    import time as _t
    runner = _CACHE["runner"]
    base_args, tbl_dev, hst_dev, wmats = _CACHE["bench"]
    wm = (wmats * ((nlayer + NLAYER - 1) // NLAYER))[:nlayer]
    t0 = _t.perf_counter()
    runner.run_layers(base_args, tbl_dev, hst_dev, wm, nlayer=nlayer)
    return _t.perf_counter() - t0
